# revision 1
# baseline (speedup 1.0000x reference)
"""Distributed causal self-attention kernel for 8 TRN2 NeuronCores.

Problem (hardcoded): B=4, T=2048, C=1024, H=16 heads, D=64 head dim, fp32.
  y = softmax(causal(x Wq^T (x Wk^T)^T / sqrt(D))) (x Wv^T) Wp^T + biases

Sharding: data-parallel over B (4 groups) x tensor-parallel over heads
(2 groups of 8 heads).  Core c handles batch c//2, head-group c%2.  Each
core computes a partial output projection y_partial = O_g @ Wp[:,cols_g]^T;
the host sums the two partials of each batch pair (the 2-way all-reduce of
the sharding hint) and adds bp.

Per-core kernel:
  phase 1: project Q/K/V from DMA'd x^T / W^T chunks (fp32 bitcast to
           f32r for full PE rate, no cast copies).  Q is evicted (DVE,
           bias fused) straight into zero-padded per-(pair,half) bf16
           stores so the S matmuls can contract K=128 (full PE array,
           keeps HAM at full clock) with no per-tile padding copies.
           K^T packed bf16; V bf16 with ones columns at [0] and [65]
           so either head-half's O matmul carries the softmax
           denominator in a PSUM row adjacent to its 64 data rows.
  phase 2: qt-outer / pair-inner flash attention.  Per 128-key chunk:
           two S^T matmuls (bf16), one 3D-AP Exp on ScalarE (both
           halves, scale=1/8 fused, bf16 out), causal mask via a
           gpsimd.affine_select restricted to the 128-column diagonal
           strip, and the O^T accumulation delayed by one chunk so the
           in-order PE never waits on the exp (the S of chunk k+1 runs
           while ScalarE exps chunk k).  The delay pipeline crosses
           pair boundaries.  Normalization is fully off the PE/ScalarE
           critical path: DVE reciprocal_approx_fast on the denominator
           row, gpsimd partition_broadcast, DVE multiply into O^T.
  phase 3: per qt (interleaved with phase 2): y[t,:] = O^T.T @ Wp_g^T
           in f32r, DVE eviction, DMA out.
"""

import numpy as np

import concourse.bass as bass
import concourse.mybir as mybir
from concourse.tile import TileContext
from concourse.bass_utils import run_bass_kernel_spmd

F32 = mybir.dt.float32
F32R = mybir.dt.float32r
BF16 = mybir.dt.bfloat16
AF = mybir.ActivationFunctionType
ALU = mybir.AluOpType

P = 128          # partitions
T = 2048         # sequence length
C = 1024         # model dim
D = 64           # head dim
HG = 8           # heads per core
J = HG * D       # per-core projection width (512)
CC = C // P      # contraction chunks over model dim (8)
JC = J // P      # j chunks (4)
NT = T // P      # 128-row t tiles (16)
TBS = 256        # t block size for x^T staging
NTB = T // TBS   # t blocks (8)
NQ = T // 512    # 512-wide query tiles (4)
NPAIR = HG // 2  # head pairs (4); pair p covers heads 2p, 2p+1

_CACHE = {}


def _split_excess_waits(nc):
    """Walrus in this container only accepts 1 sync-wait on CTRL-queue
    instructions (Drain etc.).  Hoist excess waits onto preceding nops on
    the same engine queue (program order makes this equivalent)."""
    n = 0
    for f in nc.m.functions:
        for bb in f.blocks:
            out = []
            for inst in bb.instructions:
                si = inst.sync_info
                limit = 1
                if si is not None and si.on_wait and len(si.on_wait) > limit:
                    waits = list(si.on_wait)
                    excess, keep = waits[:-limit], waits[-limit:]
                    for ci in range(0, len(excess), limit):
                        n += 1
                        out.append(mybir.InstNoOp(
                            name=f"waitsplit_{n}", opcode="nop", engine=inst.engine,
                            sync_info=mybir.SyncInfo(
                                on_wait=excess[ci:ci + limit], on_update=[]),
                        ))
                    inst.sync_info = mybir.SyncInfo(
                        on_wait=keep, on_update=list(si.on_update))
                out.append(inst)
            bb.instructions = out


def _build():
    nc = bass.Bass()
    # host passes x and the weight shards pre-transposed (layout marshaling
    # done during sharding): xt = x[b].T, w?t = W?[rows].T, wpt = Wp[:,cols].T
    xt_in = nc.dram_tensor("xt", [C, T], F32, kind="ExternalInput")
    wq_in = nc.dram_tensor("wqt", [C, J], F32, kind="ExternalInput")
    wk_in = nc.dram_tensor("wkt", [C, J], F32, kind="ExternalInput")
    wv_in = nc.dram_tensor("wvt", [C, J], F32, kind="ExternalInput")
    wp_in = nc.dram_tensor("wpt", [J, C], F32, kind="ExternalInput")
    bq_in = nc.dram_tensor("bq", [J], F32, kind="ExternalInput")
    bk_in = nc.dram_tensor("bk", [J], F32, kind="ExternalInput")
    bv_in = nc.dram_tensor("bv", [J], F32, kind="ExternalInput")
    y_out = nc.dram_tensor("y", [T, C], F32, kind="ExternalOutput")

    with TileContext(nc) as tc:
        with tc.tile_pool(name="persist", bufs=1) as persist:
            # Q padded per (pair, half): the other head's 64 partitions are
            # zero so S can contract the full 128-row pair block.
            qp_t = persist.tile([P, NPAIR, 2, T], BF16, tag="qp")
            kt_t = persist.tile([P, JC, T], BF16, tag="kt")     # K^T packed
            # V rows per (t-tile, head): [v0..v63, 1]; the ones column makes
            # PSUM row 64 of each half's O accumulation the softmax
            # denominator.
            v_t = persist.tile([P, NT, HG, D + 1], BF16, tag="v")
            ot_r = persist.tile([P, JC, T], BF16, tag="ot")     # O^T normalized
            bq_sb = persist.tile([P, JC], F32, tag="bq")
            bk_sb = persist.tile([P, JC], F32, tag="bk")
            bv_sb = persist.tile([1, J], F32, tag="bv")
            bv_r = persist.tile([1, J], F32R, tag="bvr")
            bv_bc = persist.tile([P, J], F32, tag="bvbc")
            ones_bf = persist.tile([P, P], BF16, tag="ones_bf")
            ones_row = persist.tile([1, P], F32R, tag="ones_row")

            # constants / zero-fills (gpsimd queue; PE warmup only needs
            # ones_bf so it is emitted first)
            nc.gpsimd.memset(ones_bf[:], 1.0)
            nc.vector.tensor_copy(ones_row[:], ones_bf[0:1, :])
            nc.sync.dma_start(bq_sb[:], bq_in.rearrange("(o p) -> p o", p=P))
            nc.sync.dma_start(bk_sb[:], bk_in.rearrange("(o p) -> p o", p=P))
            nc.sync.dma_start(bv_sb[:], bv_in[None, :])

            # ---------------- phase 1: QKV projections ----------------------
            with (
                tc.tile_pool(name="nat", bufs=4) as nat_pool,
                tc.tile_pool(name="xt", bufs=2) as xt_pool,
                tc.tile_pool(name="wt", bufs=1) as wt_pool,
                tc.tile_pool(name="ps_mm", bufs=3, space="PSUM") as ps_mm,
            ):
                # HAM warm-up: keep the PE streaming while the first DMAs
                # land so the clock is ramped when projections start.
                ps_warm = ps_mm.tile([P, J], F32, tag="mm", name="ps_warm")
                for _ in range(28):
                    nc.tensor.matmul(ps_warm[:, 0:P], lhsT=ones_bf[:],
                                     rhs=ones_bf[:], start=True, stop=True)

                # bv broadcast to all partitions via K=1 matmul
                nc.vector.tensor_copy(bv_r[:], bv_sb[:])
                ps_bv = ps_mm.tile([P, J], F32, tag="mm", name="ps_bv")
                nc.tensor.matmul(ps_bv[:], lhsT=ones_row[:], rhs=bv_r[:],
                                 start=True, stop=True)
                nc.vector.tensor_copy(bv_bc[:], ps_bv[:])

                # weights + x^T tiles: raw fp32 DMA into a small staging
                # tile, then the otherwise-idle ScalarE rounds to f32r (the
                # BIR verifier requires f32r matmul inputs to come from a
                # rounding instruction).  Emission order sets DMA priority:
                # wq and the first x block first, wk/wv behind them.
                wt_q = wt_pool.tile([P, CC, J], F32R, tag="wq", name="wt_q")
                wt_k = wt_pool.tile([P, CC, J], F32R, tag="wk", name="wt_k")
                wt_v = wt_pool.tile([P, CC, J], F32R, tag="wv", name="wt_v")

                def stage(dst, src_ap, name, eng=None):
                    raw = nat_pool.tile([P, J], F32, tag="nat", name=name)
                    w = src_ap.shape[-1]
                    nc.sync.dma_start(raw[:, 0:w], src_ap)
                    if eng is None:
                        nc.scalar.copy(dst, raw[:, 0:w])
                    else:
                        eng.tensor_copy(dst, raw[:, 0:w])

                def load_xt(tb):
                    # x casts alternate ScalarE / gpsimd so neither queue
                    # serializes the matmul-critical rounding
                    xt = xt_pool.tile([P, CC, TBS], F32R, tag="xt",
                                      name=f"xt_{tb}")
                    for cc in range(CC):
                        stage(xt[:, cc, :],
                              xt_in[cc * P:(cc + 1) * P,
                                    tb * TBS:(tb + 1) * TBS],
                              f"xraw_{tb}_{cc}",
                              eng=nc.gpsimd if cc % 2 else None)
                    return xt

                xt0 = xt_pool.tile([P, CC, TBS], F32R, tag="xt", name="xt_0")
                for cc in range(CC):
                    stage(wt_q[:, cc, :], wq_in[cc * P:(cc + 1) * P, :],
                          f"wqraw_{cc}", eng=nc.gpsimd if cc % 2 else None)
                    stage(xt0[:, cc, :], xt_in[cc * P:(cc + 1) * P, 0:TBS],
                          f"xraw_0_{cc}", eng=None if cc % 2 else nc.gpsimd)
                for cc in range(CC):
                    stage(wt_k[:, cc, :], wk_in[cc * P:(cc + 1) * P, :],
                          f"wkraw_{cc}")
                for cc in range(CC):
                    stage(wt_v[:, cc, :], wv_in[cc * P:(cc + 1) * P, :],
                          f"wvraw_{cc}")

                for tb in range(NTB):
                    xt = xt0 if tb == 0 else load_xt(tb)
                    tbs = slice(tb * TBS, (tb + 1) * TBS)

                    # Q^T / K^T [j, t] per j-chunk (pair)
                    for name, wt, bias in (("q", wt_q, bq_sb), ("k", wt_k, bk_sb)):
                        for jc in range(JC):
                            psq = ps_mm.tile([P, J], F32, tag="mm",
                                             name=f"ps_{name}_{tb}_{jc}")
                            for cc in range(CC):
                                nc.tensor.matmul(
                                    psq[:, 0:TBS],
                                    lhsT=wt[:, cc, jc * P:(jc + 1) * P],
                                    rhs=xt[:, cc, :],
                                    start=(cc == 0), stop=(cc == CC - 1))
                            if name == "q":
                                nc.vector.tensor_scalar_add(
                                    qp_t[0:D, jc, 0, tbs], psq[0:D, 0:TBS],
                                    bias[0:D, jc:jc + 1])
                                nc.vector.tensor_scalar_add(
                                    qp_t[D:P, jc, 1, tbs], psq[D:P, 0:TBS],
                                    bias[D:P, jc:jc + 1])
                            else:
                                nc.vector.tensor_scalar_add(
                                    kt_t[:, jc, tbs], psq[:, 0:TBS],
                                    bias[:, jc:jc + 1])

                    # V[t, j] (+ bias broadcast over t)
                    for sub in range(TBS // P):
                        tt = tb * (TBS // P) + sub
                        psv = ps_mm.tile([P, J], F32, tag="mm",
                                         name=f"ps_v_{tt}")
                        for cc in range(CC):
                            nc.tensor.matmul(
                                psv[:],
                                lhsT=xt[:, cc, sub * P:(sub + 1) * P],
                                rhs=wt_v[:, cc, :],
                                start=(cc == 0), stop=(cc == CC - 1))
                        nc.vector.tensor_tensor(
                            v_t[:, tt, :, 0:D],
                            psv.rearrange("p (h d) -> p h d", h=HG),
                            bv_bc.rearrange("p (h d) -> p h d", h=HG),
                            ALU.add)

                # zero the dead half of each Q store; ones columns of V.
                # Emitted last so the gpsimd queue serves the phase-1 x
                # casts first; these finish during the phase-1 tail.
                nc.gpsimd.memset(qp_t[0:D, :, 1, :], 0.0)
                nc.gpsimd.memset(qp_t[D:P, :, 0, :], 0.0)
                nc.gpsimd.memset(v_t[:, :, :, D:D + 1], 1.0)

            # ---------------- phases 2+3 -----------------------------------
            with (
                tc.tile_pool(name="wpt", bufs=1) as wpt_pool,
                tc.tile_pool(name="e", bufs=6) as e_pool,
                tc.tile_pool(name="rc", bufs=2) as rc_pool,
                tc.tile_pool(name="tmp", bufs=2) as tmp_pool,
                tc.tile_pool(name="yout", bufs=2) as y_pool,
                tc.tile_pool(name="ps_s", bufs=2, space="PSUM") as ps_s,
                tc.tile_pool(name="ps_o", bufs=2, space="PSUM") as ps_o,
            ):
                # Wp^T (host pre-transposed) staged raw, cast to bf16 on DVE
                wpt = wpt_pool.tile([P, JC, C], BF16, tag="wpt")
                for jc in range(JC):
                    wraw = tmp_pool.tile([P, C], F32, tag="wraw",
                                         name=f"wraw_{jc}")
                    nc.sync.dma_start(wraw[:], wp_in[jc * P:(jc + 1) * P, :])
                    nc.vector.tensor_copy(wpt[:, jc, :], wraw[:])

                def emit_o(pend):
                    pso, pair, qt, kc, nk, e, delta = pend
                    for half in range(2):
                        h = pair * 2 + half
                        nc.tensor.matmul(
                            pso[0:D + 1, half, delta:],
                            lhsT=v_t[:, kc, h, :],
                            rhs=e[:, half, delta:],
                            start=(kc == 0), stop=(kc == nk - 1))

                def emit_norm_pre(pend):
                    # 1/denominator as exp(-ln d) on ScalarE: ln and exp
                    # live in the same activation table (no table swaps),
                    # and the 37%-busy ScalarE absorbs ~2.3us/pair easily
                    # (DVE's InstReciprocal was 3.3us/half and walled the
                    # in-order DVE queue at qt boundaries).
                    pso, pair, qt, kc, nk, e, delta = pend
                    lnd = rc_pool.tile([1, 2, 512], F32, tag="lnd",
                                       name=f"lnd_{qt}_{pair}")
                    nc.scalar.activation(lnd[:], pso[D:D + 1, :, :], AF.Ln)
                    rcr = rc_pool.tile([1, 2, 512], F32R, tag="rcr",
                                       name=f"rcr_{qt}_{pair}")
                    nc.scalar.activation(rcr[:], lnd[:], AF.Exp, scale=-1.0)
                    return rcr

                def emit_norm_post(pend, rcr):
                    # K=1 matmul broadcast of 1/denom, then DVE multiplies
                    # into normalized O^T.
                    pso, pair, qt, kc, nk, e, delta = pend
                    qs = slice(qt * 512, (qt + 1) * 512)
                    psb = ps_s.tile([P, 2, 512], F32, tag="s",
                                    name=f"psb_{qt}_{pair}")
                    for half in range(2):
                        nc.tensor.matmul(
                            psb[:, half, :], lhsT=ones_row[:],
                            rhs=rcr[:, half, :], start=True, stop=True)
                    # one PSUM operand max per DVE op: stage bc in SBUF
                    bc = tmp_pool.tile([D, 2, 512], F32, tag="bc",
                                       name=f"bc_{qt}_{pair}")
                    nc.vector.tensor_copy(bc[:], psb[0:D, :, :])
                    nc.vector.tensor_tensor(
                        ot_r[0:D, pair, qs], pso[0:D, 0, :],
                        bc[:, 0, :], ALU.mult)
                    # normalized upper half staged in SBUF, then a
                    # cross-partition DMA into O^T rows 64:128
                    tmp = tmp_pool.tile([D, 512], BF16, tag="tmp",
                                        name=f"tmp_{qt}_{pair}")
                    nc.vector.tensor_tensor(
                        tmp[:], pso[0:D, 1, :], bc[:, 1, :], ALU.mult)
                    nc.sync.dma_start(ot_r[D:P, pair, qs], tmp[:])

                def phase3_tile(tt):
                    ts = slice(tt * P, (tt + 1) * P)
                    psy = ps_o.tile([P, 2, 512], F32, tag="o",
                                    name=f"psy_{tt}")
                    for nh in range(2):
                        for jc in range(JC):
                            nc.tensor.matmul(
                                psy[:, nh, :],
                                lhsT=ot_r[:, jc, ts],
                                rhs=wpt[:, jc, nh * 512:(nh + 1) * 512],
                                start=(jc == 0), stop=(jc == JC - 1))
                    ytile = y_pool.tile([P, C], F32, tag="y",
                                        name=f"y_{tt}")
                    nc.vector.tensor_copy(
                        ytile.rearrange("p (n q) -> p n q", n=2), psy[:])
                    nc.sync.dma_start(y_out[ts, :], ytile[:])

                from collections import deque
                pendq = deque()  # chunks awaiting their O matmuls (depth 2)
                actions = []     # [countdown, fn]: deferred norm/phase-3
                                 # work woven into later chunks

                def tick():
                    fire = []
                    for a in actions:
                        a[0] -= 1
                        if a[0] <= 0:
                            fire.append(a)
                    for a in fire:
                        actions.remove(a)
                        a[1]()

                def fire_o(pend):
                    emit_o(pend)
                    if pend[3] == pend[4] - 1:  # last chunk of its pair
                        rcr = emit_norm_pre(pend)
                        actions.append(
                            [4, (lambda a, b: lambda:
                                 emit_norm_post(a, b))(pend, rcr)])

                for qt in range(NQ):
                    nk = (qt + 1) * 4
                    for pair in range(NPAIR):
                        pso = ps_o.tile([P, 2, 512], F32, tag="o",
                                        name=f"pso_{qt}_{pair}")
                        for kc in range(nk):
                            # columns q < delta of this chunk are fully
                            # masked; skip them in S, exp and O.
                            delta = max(0, (kc - qt * 4) * P)
                            ks = slice(kc * P, (kc + 1) * P)
                            pss = ps_s.tile([P, 2, 512], F32, tag="s",
                                            name=f"pss_{qt}_{pair}_{kc}")
                            for half in range(2):
                                nc.tensor.matmul(
                                    pss[:, half, delta:],
                                    lhsT=kt_t[:, pair, ks],
                                    rhs=qp_t[:, pair, half,
                                             qt * 512 + delta:(qt + 1) * 512],
                                    start=True, stop=True)
                            e = e_pool.tile([P, 2, 512], BF16, tag="e",
                                            name=f"e_{qt}_{pair}_{kc}")
                            nc.scalar.activation(
                                e[:, :, delta:], pss[:, :, delta:],
                                AF.Exp, scale=0.125)
                            if kc >= qt * 4:  # diagonal: mask the 128-col strip
                                nc.gpsimd.affine_select(
                                    out=e[:, :, delta:delta + P],
                                    in_=e[:, :, delta:delta + P],
                                    compare_op=ALU.is_ge, fill=0.0,
                                    base=0, channel_multiplier=-1,
                                    pattern=[[0, 2], [1, P]])
                            tick()
                            if len(pendq) == 2:
                                fire_o(pendq.popleft())
                            pendq.append((pso, pair, qt, kc, nk, e, delta))
                    # qt done: flush pending O matmuls; the last pair's
                    # normalize and this qt's output projection are woven
                    # into the next qt's chunk stream.
                    while pendq:
                        fire_o(pendq.popleft())
                    actions.extend(
                        [8 + i, (lambda t: lambda: phase3_tile(t))(tt)]
                        for i, tt in enumerate(range(qt * 4, qt * 4 + 4)))

                # drain: fire remaining deferred work in order
                for a in sorted(actions, key=lambda a: a[0]):
                    a[1]()

    _split_excess_waits(nc)
    return nc


def _get_nc():
    if "nc" not in _CACHE:
        _CACHE["nc"] = _build()
    return _CACHE["nc"]


def kernel(x, Wq, bq, Wk, bk, Wv, bv, Wp, bp, **_unused):
    x = np.ascontiguousarray(np.asarray(x, dtype=np.float32))
    Wq = np.asarray(Wq, dtype=np.float32)
    Wk = np.asarray(Wk, dtype=np.float32)
    Wv = np.asarray(Wv, dtype=np.float32)
    Wp = np.asarray(Wp, dtype=np.float32)
    bq = np.asarray(bq, dtype=np.float32)
    bk = np.asarray(bk, dtype=np.float32)
    bv = np.asarray(bv, dtype=np.float32)
    bp = np.asarray(bp, dtype=np.float32)

    nc = _get_nc()
    in_maps = []
    for c in range(8):
        b, g = c // 2, c % 2
        js = slice(g * J, (g + 1) * J)
        in_maps.append({
            "xt": np.ascontiguousarray(x[b].T),
            "wqt": np.ascontiguousarray(Wq[js, :].T),
            "wkt": np.ascontiguousarray(Wk[js, :].T),
            "wvt": np.ascontiguousarray(Wv[js, :].T),
            "wpt": np.ascontiguousarray(Wp[:, js].T),
            "bq": np.ascontiguousarray(bq[js]),
            "bk": np.ascontiguousarray(bk[js]),
            "bv": np.ascontiguousarray(bv[js]),
        })
    res = run_bass_kernel_spmd(nc, in_maps, list(range(8)))
    out = np.empty((4, T, C), dtype=np.float32)
    for b in range(4):
        out[b] = res.results[2 * b]["y"] + res.results[2 * b + 1]["y"] + bp
    return out

